# revision 20
# baseline (speedup 1.0000x reference)
"""CommNet critic forward kernel for 8 trn2 NeuronCores.

Sharding: pure data parallel over the batch dim (B=2048 -> 256 per core).
Weights (<1MB) replicated. The agent-mean communication is within each
sample's 32-agent group, which never crosses a core boundary, so there are
no collectives.

On-chip layout is feature-major: activations live as [feature -> partition,
row -> free-dim] tiles, so every matmul is out = W_T.T @ acts with the
weight stationary, and the per-sample mean over 32 agents is a free-dim
segmented reduction. obs is transposed host-side so the device never
transposes anything.

v4 notes (engine balance + pipeline):
  * fobs_W folded into the GRU1 gh matmul (both linear, no activation
    between), so h0 never feeds a matmul; its PSUM is consumed by the
    d1 = (ph + fobs_b) - n1 scalar_tensor_tensor, fobs bias fused.
  * ACT keeps sigmoid/tanh + half the enc relu; DVE keeps the PSUM-side
    stt chain; GpSimd takes the GRU2 update and the comm broadcast-sub;
    the h2 = n2 + m2 add rides a SWDGE accumulate-DMA (CCE) so the
    GpSimd queue stays short.
  * PSUM: pe/ph/pn share a 2-slot pool of [128,1024] tiles (4 banks),
    everything else rotates through a 4-slot [128,512] pool; allocation
    order is arranged so slot-reuse waits land on early ACT sigmoids,
    not on late DVE stt products.
  * Decoder trails B by 3 tiles so its matmuls never wait on the GpSimd
    tail; its PSUM->SBUF move alternates ACT/DVE per tile.
  * Input x tiles for the prologue are prefetched before the big weight
    DMAs so the first matmul starts ~2us in, not ~15us.

All matmul operands are bf16 (fp32 PSUM accumulate): the fast PE path.
"""

import sys

sys.path.insert(0, "/opt/trn_rl_repo")

import ml_dtypes
import numpy as np

import concourse.bacc as bacc
import concourse.mybir as mybir
import concourse.tile as tile
from concourse.bass_utils import run_bass_kernel_spmd

B, A, D, H = 2048, 32, 128, 256
NCORES = 8
B_LOC = B // NCORES          # 256 samples per core
N_LOC = B_LOC * A            # 8192 rows per core
R = 512                      # rows per tile (one PSUM bank of fp32)

F32 = mybir.dt.float32
BF16 = mybir.dt.bfloat16
NP_BF16 = ml_dtypes.bfloat16

AF = mybir.ActivationFunctionType
OP = mybir.AluOpType

S0 = slice(0, 512)
S1 = slice(512, 1024)


def build_nc(n_rows=N_LOC):
    assert n_rows % R == 0
    nt = n_rows // R
    nc = bacc.Bacc("TRN2", target_bir_lowering=False, debug=False)

    xT = nc.declare_dram_parameter("xT", [D, n_rows], BF16, isOutput=False)
    encT = nc.declare_dram_parameter("encT", [128, 256], BF16, isOutput=False)
    fobsT = nc.declare_dram_parameter("fobsT", [128, 512], BF16, isOutput=False)
    wf1T = nc.declare_dram_parameter("wf1T", [128, 1536], BF16, isOutput=False)
    whhT = nc.declare_dram_parameter("whhT", [128, 1536], BF16, isOutput=False)
    wihT = nc.declare_dram_parameter("wihT", [128, 1536], BF16, isOutput=False)
    decT = nc.declare_dram_parameter("decT", [128, 2], BF16, isOutput=False)
    encb = nc.declare_dram_parameter("encb", [128, 2], F32, isOutput=False)
    fobsb = nc.declare_dram_parameter("fobsb", [128, 2], F32, isOutput=False)
    brz1 = nc.declare_dram_parameter("brz1", [128, 4], F32, isOutput=False)
    bhn1 = nc.declare_dram_parameter("bhn1", [128, 2], F32, isOutput=False)
    binb = nc.declare_dram_parameter("binb", [128, 2], F32, isOutput=False)
    brz2 = nc.declare_dram_parameter("brz2", [128, 4], F32, isOutput=False)
    bhn2 = nc.declare_dram_parameter("bhn2", [128, 2], F32, isOutput=False)
    decb = nc.declare_dram_parameter("decb", [1, 1], F32, isOutput=False)
    out = nc.declare_dram_parameter("out", [1, n_rows], F32, isOutput=True)

    def mm(o, lhsT, rhs, start, stop):
        nc.tensor.matmul(o, lhsT, rhs, start=start, stop=stop)

    with tile.TileContext(nc, pool_alloc_mode="queue") as tc:
        with (
            tc.tile_pool(name="wpool", bufs=1) as wp,
            tc.tile_pool(name="io", bufs=4) as io,
            tc.tile_pool(name="acts", bufs=4) as ap,
            tc.tile_pool(name="psum", bufs=4, space="PSUM") as pp,
            tc.tile_pool(name="psumx", bufs=2, space="PSUM") as px,
        ):
            encT_s = wp.tile([128, 256], BF16, name="encT_s", tag="encT_s")
            fobsT_s = wp.tile([128, 512], BF16, name="fobsT_s", tag="fobsT_s")
            wf1T_s = wp.tile([128, 1536], BF16, name="wf1T_s", tag="wf1T_s")
            whhT_s = wp.tile([128, 1536], BF16, name="whhT_s", tag="whhT_s")
            wihT_s = wp.tile([128, 1536], BF16, name="wihT_s", tag="wihT_s")
            decT_s = wp.tile([128, 2], BF16, name="decT_s", tag="decT_s")
            encb_s = wp.tile([128, 2], F32, name="encb_s", tag="encb_s")
            fobsb_s = wp.tile([128, 2], F32, name="fobsb_s", tag="fobsb_s")
            brz1_s = wp.tile([128, 4], F32, name="brz1_s", tag="brz1_s")
            bhn1_s = wp.tile([128, 2], F32, name="bhn1_s", tag="bhn1_s")
            binb_s = wp.tile([128, 2], F32, name="binb_s", tag="binb_s")
            brz2_s = wp.tile([128, 4], F32, name="brz2_s", tag="brz2_s")
            bhn2_s = wp.tile([128, 2], F32, name="bhn2_s", tag="bhn2_s")
            decb_s = wp.tile([1, 1], F32, name="decb_s", tag="decb_s")

            xT_ap = xT.ap()
            out_ap = out.ap()

            # Prefetch the prologue input tiles ahead of the big weight
            # DMAs (same SP queue) so the first enc matmul starts early.
            nc.sync.dma_start(encT_s[:], encT.ap())
            nc.sync.dma_start(encb_s[:], encb.ap())
            xt_pre = {}
            for t in range(4):
                xt = io.tile([128, R], BF16, name="xt", tag="xt")
                # ACT is also a HWDGE engine: its queue runs in parallel
                # with the SP weight DMAs, halving the ramp.
                nc.scalar.dma_start(xt[:], xT_ap[:, t * R : t * R + R])
                xt_pre[t] = xt
            for tl, dr in [
                (fobsT_s, fobsT), (wf1T_s, wf1T), (brz1_s, brz1),
                (bhn1_s, bhn1), (binb_s, binb), (fobsb_s, fobsb),
                (whhT_s, whhT), (wihT_s, wihT), (brz2_s, brz2),
                (bhn2_s, bhn2), (decT_s, decT), (decb_s, decb),
            ]:
                nc.sync.dma_start(tl[:], dr.ap())

            st = {}

            # ---- A phases: enc -> e; gh1 = Wf1@e; ph = fobs@e ----

            def emitA1(t):
                # enc: e = relu(enc_W @ x + enc_b); one half ACT, one DVE
                if t in xt_pre:
                    xt = xt_pre.pop(t)
                else:
                    r0 = t * R
                    xt = io.tile([128, R], BF16, name="xt", tag="xt")
                    nc.sync.dma_start(xt[:], xT_ap[:, r0 : r0 + R])
                pe = px.tile([128, 1024], F32, name="pe", tag="px")
                for m in (0, 1):
                    mm(pe[:, 512 * m : 512 * m + 512],
                       encT_s[:, 128 * m : 128 * m + 128], xt[:], True, True)
                e = ap.tile([128, 1024], BF16, name="e", tag="e", bufs=5)
                for m, sl in ((0, S0), (1, S1)):
                    nc.scalar.activation(e[:, sl], pe[:, sl], AF.Relu,
                                         bias=encb_s[:, m : m + 1])
                st[t] = {"e": e}

            def emitA2(t):
                # fobs ph (PSUM, consumed by d1) + gh1 = Wf1 @ e
                e = st[t]["e"]
                # pn allocated before ph: the px FIFO then couples next
                # tile's pe slot to tmp1 (early DVE) instead of d1 (late).
                pn = px.tile([128, 1024], F32, name="pn", tag="px")
                ph = px.tile([128, 1024], F32, name="ph", tag="px")
                prz = [pp.tile([128, 512], F32, name=f"prz{g}", tag="ps")
                       for g in range(4)]
                # k-outer: all k=0 matmuls (reading the ACT-produced e half)
                # run before any k=1 (DVE-produced half), so a late relu on
                # the DVE queue never stalls the PE.
                for k in (0, 1):
                    ek = e[:, 512 * k : 512 * k + 512]
                    for m in (0, 1):
                        mm(ph[:, 512 * m : 512 * m + 512],
                           fobsT_s[:, 256 * k + 128 * m : 256 * k + 128 * m + 128],
                           ek, k == 0, k == 1)
                    for m in (0, 1):
                        mm(pn[:, 512 * m : 512 * m + 512],
                           wf1T_s[:, 768 * k + 512 + 128 * m : 768 * k + 640 + 128 * m],
                           ek, k == 0, k == 1)
                    for g in range(4):
                        mm(prz[g][:],
                           wf1T_s[:, 768 * k + 128 * g : 768 * k + 128 * g + 128],
                           ek, k == 0, k == 1)
                st[t]["ph"] = ph
                st[t]["pn"] = pn
                st[t]["prz"] = prz

            def emitA3a(t):
                # GRU1 sigmoids (ACT)
                prz = st[t].pop("prz")
                rz1 = ap.tile([128, 2048], BF16, name="rz1", tag="rz1", bufs=5)
                for g in range(4):
                    nc.scalar.activation(rz1[:, 512 * g : 512 * g + 512],
                                         prz[g][:], AF.Sigmoid,
                                         bias=brz1_s[:, g : g + 1])
                st[t]["rz1"] = rz1

            def emitA3bHead(t):
                # DVE/ACT: tmp1, n1, d1 (fobs PSUM, bias fused). The tail
                # (m1/h1/S) is emitted one step later, when its inputs are
                # already resident, so it never clogs the DVE queue.
                pn = st[t].pop("pn")
                ph = st[t].pop("ph")
                rz1 = st[t]["rz1"]
                tmp1 = ap.tile([128, 1024], BF16, name="tmp1", tag="tmp1")
                for m, sl in ((0, S0), (1, S1)):
                    nc.vector.scalar_tensor_tensor(
                        tmp1[:, sl], pn[:, sl], bhn1_s[:, m : m + 1],
                        rz1[:, sl], OP.add, OP.mult)
                n1 = ap.tile([128, 1024], BF16, name="n1", tag="n1")
                for m, sl in ((0, S0), (1, S1)):
                    nc.scalar.activation(n1[:, sl], tmp1[:, sl], AF.Tanh,
                                         bias=binb_s[:, m : m + 1])
                d1 = ap.tile([128, 1024], BF16, name="d1", tag="d1")
                for m, sl in ((0, S0), (1, S1)):
                    nc.vector.scalar_tensor_tensor(
                        d1[:, sl], ph[:, sl], fobsb_s[:, m : m + 1],
                        n1[:, sl], OP.add, OP.subtract)
                st[t]["n1"] = n1
                st[t]["d1"] = d1

            def emitA3bTail(t):
                # m1, h1 (DVE); agent pair-add (GpSimd); short reduce (DVE)
                rz1 = st[t].pop("rz1")
                n1 = st[t].pop("n1")
                d1 = st[t].pop("d1")
                m1 = ap.tile([128, 1024], BF16, name="m1", tag="m1")
                nc.vector.tensor_mul(m1[:], rz1[:, 1024:2048], d1[:])
                h1 = ap.tile([128, 1024], BF16, name="h1", tag="h1", bufs=7)
                nc.vector.tensor_add(h1[:], n1[:], m1[:])
                h1v = h1[:].rearrange("p (s two a) -> p two s a", two=2, a=16)
                s2t = ap.tile([128, 512], BF16, name="s2t", tag="s2t")
                nc.gpsimd.tensor_tensor(
                    s2t[:].rearrange("p (s a) -> p s a", a=16),
                    h1v[:, 0:1, :, :].rearrange("p one s a -> p (one s) a"),
                    h1v[:, 1:2, :, :].rearrange("p one s a -> p (one s) a"),
                    OP.add)
                S = ap.tile([128, 32], F32, name="S", tag="S")
                nc.vector.tensor_reduce(
                    S[:], s2t[:].rearrange("p (s a) -> p s a", a=16),
                    mybir.AxisListType.X, OP.add)
                st[t]["h1"] = h1
                st[t]["S"] = S

            def emitA3c(t):
                # comm: c' = (sum_group h1) - h1  (1/A folded into W_ih)
                h1, S = st[t]["h1"], st[t].pop("S")
                cp = ap.tile([128, 1024], BF16, name="cp", tag="cp", bufs=7)
                Sb = S[:].unsqueeze(-1).broadcast_to([128, 32, 32])
                nc.gpsimd.tensor_tensor(
                    cp[:].rearrange("p (s a) -> p s a", a=32), Sb,
                    h1[:].rearrange("p (s a) -> p s a", a=32), OP.subtract)
                st[t]["cp"] = cp

            # ---- B phases: GRU2 ----

            def emitBrz(t):
                h1, cp = st[t]["h1"], st[t]["cp"]
                prz2 = [pp.tile([128, 512], F32, name=f"prz2{g}", tag="ps")
                        for g in range(4)]
                for g in range(4):
                    w0 = 128 * g
                    mm(prz2[g][:], wihT_s[:, w0 : w0 + 128], cp[:, S0], True, False)
                    mm(prz2[g][:], wihT_s[:, 768 + w0 : 768 + w0 + 128], cp[:, S1], False, False)
                    mm(prz2[g][:], whhT_s[:, w0 : w0 + 128], h1[:, S0], False, False)
                    mm(prz2[g][:], whhT_s[:, 768 + w0 : 768 + w0 + 128], h1[:, S1], False, True)
                rz2 = ap.tile([128, 2048], BF16, name="rz2", tag="rz2")
                for g in range(4):
                    nc.scalar.activation(rz2[:, 512 * g : 512 * g + 512],
                                         prz2[g][:], AF.Sigmoid,
                                         bias=brz2_s[:, g : g + 1])
                st[t]["rz2"] = rz2

            def emitBnMM(t):
                h1, cp = st[t]["h1"], st[t]["cp"]
                phn = [pp.tile([128, 512], F32, name=f"phn{m}", tag="ps")
                       for m in (0, 1)]
                for m in (0, 1):
                    for k in (0, 1):
                        mm(phn[m][:],
                           whhT_s[:, 768 * k + 512 + 128 * m : 768 * k + 640 + 128 * m],
                           h1[:, 512 * k : 512 * k + 512], k == 0, k == 1)
                pin = [pp.tile([128, 512], F32, name=f"pin{m}", tag="ps")
                       for m in (0, 1)]
                for m in (0, 1):
                    for k in (0, 1):
                        mm(pin[m][:],
                           wihT_s[:, 768 * k + 512 + 128 * m : 768 * k + 640 + 128 * m],
                           cp[:, 512 * k : 512 * k + 512], k == 0, k == 1)
                st[t]["phn"] = phn
                st[t]["pin"] = pin

            def emitBelemA(t):
                # DVE: tmp2 = (phn + b_hn)*r2 ; s2 = (pin + b_in) + tmp2
                phn = st[t].pop("phn")
                pin = st[t].pop("pin")
                rz2 = st[t]["rz2"]
                tmp2 = ap.tile([128, 1024], BF16, name="tmp2", tag="tmp2")
                for m, sl in ((0, S0), (1, S1)):
                    nc.vector.scalar_tensor_tensor(
                        tmp2[:, sl], phn[m][:], bhn2_s[:, m : m + 1],
                        rz2[:, sl], OP.add, OP.mult)
                s2 = ap.tile([128, 1024], BF16, name="s2", tag="s2")
                for m, sl in ((0, S0), (1, S1)):
                    nc.vector.scalar_tensor_tensor(
                        s2[:, sl], pin[m][:], binb_s[:, m : m + 1],
                        tmp2[:, sl], OP.add, OP.add)
                st[t]["s2"] = s2

            def emitBelemB(t, on_dve=False):
                # n2 (ACT, single 1024-wide); d2/m2 on GpSimd; the final
                # h2 = n2 + m2 rides a SWDGE accumulate-DMA (n2 is written
                # straight into the h2 tile). Drain tiles go full-DVE.
                h1 = st[t]["h1"]
                s2 = st[t].pop("s2")
                rz2 = st[t].pop("rz2")
                if on_dve:
                    n2 = ap.tile([128, 1024], BF16, name="n2", tag="n2")
                    nc.scalar.activation(n2[:], s2[:], AF.Tanh)
                    d2 = ap.tile([128, 1024], BF16, name="d2", tag="d2")
                    nc.vector.tensor_tensor(d2[:], h1[:], n2[:], OP.subtract)
                    m2 = ap.tile([128, 1024], BF16, name="m2", tag="m2")
                    nc.vector.tensor_mul(m2[:], rz2[:, 1024:2048], d2[:])
                    h2 = ap.tile([128, 1024], BF16, name="h2", tag="h2", bufs=5)
                    nc.vector.tensor_add(h2[:], n2[:], m2[:])
                else:
                    h2 = ap.tile([128, 1024], BF16, name="h2", tag="h2", bufs=5)
                    nc.scalar.activation(h2[:], s2[:], AF.Tanh)  # h2 holds n2
                    d2 = ap.tile([128, 1024], BF16, name="d2", tag="d2")
                    nc.gpsimd.tensor_tensor(d2[:], h1[:], h2[:], OP.subtract)
                    m2 = ap.tile([128, 1024], BF16, name="m2", tag="m2")
                    nc.gpsimd.tensor_mul(m2[:], rz2[:, 1024:2048], d2[:])
                    nc.gpsimd.dma_start(h2[:], m2[:], accum_op=OP.add)
                st[t]["h2"] = h2

            def emitC(t):
                h2 = st.pop(t)["h2"]
                r0 = t * R
                pd = pp.tile([1, 512], F32, name="pd", tag="ps")
                mm(pd[:], decT_s[:, 0:1], h2[:, S0], True, False)
                mm(pd[:], decT_s[:, 1:2], h2[:, S1], False, True)
                ot = io.tile([1, 512], F32, name="ot", tag="ot")
                nc.scalar.activation(ot[:], pd[:], AF.Identity,
                                     bias=decb_s[0:1, 0:1])
                nc.sync.dma_start(out_ap[0:1, r0 : r0 + R], ot[:])

            # ---- schedule ----
            # A phases lead B by 3 tiles so the DVE/GpSimd chains (which
            # run up to a tile behind their emission) never gate a PSUM
            # slot reuse or a B-phase matmul; the decoder trails by 3.
            for f in (emitA1, emitA2, emitA3a, emitA3bHead,
                      emitA3bTail, emitA3c):
                f(0)
                f(1)
                f(2)
            for t in range(nt):
                if t + 3 < nt:
                    emitA1(t + 3)
                emitBrz(t)
                if t >= 3:
                    emitC(t - 3)
                emitBnMM(t)
                emitBelemA(t)
                if 3 <= t + 2 < nt:
                    emitA3bTail(t + 2)
                    emitA3c(t + 2)
                if t + 3 < nt:
                    emitA2(t + 3)
                    emitA3a(t + 3)
                    emitA3bHead(t + 3)
                emitBelemB(t, on_dve=(t >= nt - 2))
            for t in range(nt - 3, nt):
                emitC(t)

    nc.compile()
    return nc


def prep_shared(enc_W, enc_b, fobs_W, fobs_b, W_ih, b_ih, W_hh, b_hh, dec_W, dec_b):
    f = np.float32
    whh = W_hh.astype(f)
    wf1 = whh @ fobs_W.astype(f)                 # [768, 256] folded GRU1 weight
    bf1 = whh @ fobs_b.astype(f)                 # [768] folded fobs bias
    whhT = whh.T                                 # [256, 768]
    wf1T = wf1.T                                 # [256, 768]
    wihT = (W_ih / A).T.astype(f)                # [256, 768], 1/A folded in
    bsum = (b_ih + b_hh).astype(f)
    bf = NP_BF16

    def halves(mT):                              # [256, X] -> [128, 2X]
        return np.ascontiguousarray(
            np.concatenate([mT[0:128], mT[128:256]], axis=1)
        ).astype(bf)

    return {
        "encT": np.ascontiguousarray(enc_W.T).astype(bf),                    # [128,256]
        "fobsT": halves(fobs_W.T.astype(f)),                                 # [128,512]
        "wf1T": halves(wf1T),                                                # [128,1536]
        "whhT": halves(whhT),                                                # [128,1536]
        "wihT": halves(wihT),                                                # [128,1536]
        "decT": halves(dec_W.T.astype(f)),                                   # [128,2]
        "encb": np.ascontiguousarray(enc_b.reshape(2, 128).T.astype(f)),
        "fobsb": np.ascontiguousarray(fobs_b.reshape(2, 128).T.astype(f)),
        "brz1": np.ascontiguousarray((bsum[0:512] + bf1[0:512]).reshape(4, 128).T),
        "bhn1": np.ascontiguousarray(
            (b_hh[512:768] + bf1[512:768]).reshape(2, 128).T.astype(f)
        ),
        "binb": np.ascontiguousarray(b_ih[512:768].reshape(2, 128).T.astype(f)),
        "brz2": np.ascontiguousarray(bsum[0:512].reshape(4, 128).T),
        "bhn2": np.ascontiguousarray(b_hh[512:768].reshape(2, 128).T.astype(f)),
        "decb": dec_b.reshape(1, 1).astype(f),
    }


_NC_CACHE = {}


def _get_nc(n_rows):
    if n_rows not in _NC_CACHE:
        _NC_CACHE[n_rows] = build_nc(n_rows)
    return _NC_CACHE[n_rows]


def run(inputs, trace=False):
    """Shard, run on 8 cores, gather. Returns (out [B,A,1] f32, results)."""
    obs = np.asarray(inputs["obs"], dtype=np.float32)
    shared = prep_shared(
        np.asarray(inputs["enc_W"]), np.asarray(inputs["enc_b"]),
        np.asarray(inputs["fobs_W"]), np.asarray(inputs["fobs_b"]),
        np.asarray(inputs["W_ih"]), np.asarray(inputs["b_ih"]),
        np.asarray(inputs["W_hh"]), np.asarray(inputs["b_hh"]),
        np.asarray(inputs["dec_W"]), np.asarray(inputs["dec_b"]),
    )
    in_maps = []
    for c in range(NCORES):
        xT = np.ascontiguousarray(
            obs[c * B_LOC : (c + 1) * B_LOC].reshape(N_LOC, D).T
        ).astype(NP_BF16)
        in_maps.append({"xT": xT, **shared})

    nc = _get_nc(N_LOC)
    res = run_bass_kernel_spmd(nc, in_maps, core_ids=list(range(NCORES)), trace=trace)
    outs = [res.results[c]["out"].reshape(N_LOC) for c in range(NCORES)]
    full = np.concatenate(outs).reshape(B, A, 1).astype(np.float32)
    return full, res


def kernel(**inputs):
    out, _ = run(inputs, trace=False)
    return out


# revision 21
# speedup vs baseline: 1.3284x; 1.3284x over previous
"""CommNet critic forward kernel for 8 trn2 NeuronCores.

Sharding: pure data parallel over the batch dim (B=2048 -> 256 per core).
Weights (<1MB) replicated. The agent-mean communication is within each
sample's 32-agent group, which never crosses a core boundary, so there are
no collectives.

On-chip layout is feature-major: activations live as [feature -> partition,
row -> free-dim] tiles, so every matmul is out = W_T.T @ acts with the
weight stationary, and the per-sample mean over 32 agents is a free-dim
segmented reduction. obs is transposed host-side so the device never
transposes anything.

v4 notes (engine balance + pipeline):
  * fobs_W folded into the GRU1 gh matmul (both linear, no activation
    between), so h0 never feeds a matmul; its PSUM is consumed by the
    d1 = (ph + fobs_b) - n1 scalar_tensor_tensor, fobs bias fused.
  * ACT keeps sigmoid/tanh + half the enc relu; DVE keeps the PSUM-side
    stt chain; GpSimd takes the GRU2 update and the comm broadcast-sub;
    the h2 = n2 + m2 add rides a SWDGE accumulate-DMA (CCE) so the
    GpSimd queue stays short.
  * PSUM: pe/ph/pn share a 2-slot pool of [128,1024] tiles (4 banks),
    everything else rotates through a 4-slot [128,512] pool; allocation
    order is arranged so slot-reuse waits land on early ACT sigmoids,
    not on late DVE stt products.
  * Decoder trails B by 3 tiles so its matmuls never wait on the GpSimd
    tail; its PSUM->SBUF move alternates ACT/DVE per tile.
  * Input x tiles for the prologue are prefetched before the big weight
    DMAs so the first matmul starts ~2us in, not ~15us.

All matmul operands are bf16 (fp32 PSUM accumulate): the fast PE path.
"""

import sys

sys.path.insert(0, "/opt/trn_rl_repo")

import ml_dtypes
import numpy as np

import concourse.bacc as bacc
import concourse.mybir as mybir
import concourse.tile as tile
from concourse.bass_utils import run_bass_kernel_spmd

B, A, D, H = 2048, 32, 128, 256
NCORES = 8
B_LOC = B // NCORES          # 256 samples per core
N_LOC = B_LOC * A            # 8192 rows per core
R = 512                      # rows per tile (one PSUM bank of fp32)

F32 = mybir.dt.float32
BF16 = mybir.dt.bfloat16
NP_BF16 = ml_dtypes.bfloat16

AF = mybir.ActivationFunctionType
OP = mybir.AluOpType

S0 = slice(0, 512)
S1 = slice(512, 1024)


def build_nc(n_rows=N_LOC):
    assert n_rows % R == 0
    nt = n_rows // R
    nc = bacc.Bacc("TRN2", target_bir_lowering=False, debug=False)

    xT = nc.declare_dram_parameter("xT", [D, n_rows], BF16, isOutput=False)
    encT = nc.declare_dram_parameter("encT", [128, 256], BF16, isOutput=False)
    fobsT = nc.declare_dram_parameter("fobsT", [128, 512], BF16, isOutput=False)
    wf1T = nc.declare_dram_parameter("wf1T", [128, 1536], BF16, isOutput=False)
    whhT = nc.declare_dram_parameter("whhT", [128, 1536], BF16, isOutput=False)
    wihT = nc.declare_dram_parameter("wihT", [128, 1536], BF16, isOutput=False)
    decT = nc.declare_dram_parameter("decT", [128, 2], BF16, isOutput=False)
    encb = nc.declare_dram_parameter("encb", [128, 2], F32, isOutput=False)
    fobsb = nc.declare_dram_parameter("fobsb", [128, 2], F32, isOutput=False)
    brz1 = nc.declare_dram_parameter("brz1", [128, 4], F32, isOutput=False)
    bhn1 = nc.declare_dram_parameter("bhn1", [128, 2], F32, isOutput=False)
    binb = nc.declare_dram_parameter("binb", [128, 2], F32, isOutput=False)
    brz2 = nc.declare_dram_parameter("brz2", [128, 4], F32, isOutput=False)
    bhn2 = nc.declare_dram_parameter("bhn2", [128, 2], F32, isOutput=False)
    decb = nc.declare_dram_parameter("decb", [1, 1], F32, isOutput=False)
    out = nc.declare_dram_parameter("out", [1, n_rows], F32, isOutput=True)

    def mm(o, lhsT, rhs, start, stop):
        nc.tensor.matmul(o, lhsT, rhs, start=start, stop=stop)

    with tile.TileContext(nc, pool_alloc_mode="queue") as tc:
        with (
            tc.tile_pool(name="wpool", bufs=1) as wp,
            tc.tile_pool(name="io", bufs=4) as io,
            tc.tile_pool(name="acts", bufs=4) as ap,
            tc.tile_pool(name="psum", bufs=4, space="PSUM") as pp,
            tc.tile_pool(name="psumx", bufs=2, space="PSUM") as px,
        ):
            encT_s = wp.tile([128, 256], BF16, name="encT_s", tag="encT_s")
            fobsT_s = wp.tile([128, 512], BF16, name="fobsT_s", tag="fobsT_s")
            wf1T_s = wp.tile([128, 1536], BF16, name="wf1T_s", tag="wf1T_s")
            whhT_s = wp.tile([128, 1536], BF16, name="whhT_s", tag="whhT_s")
            wihT_s = wp.tile([128, 1536], BF16, name="wihT_s", tag="wihT_s")
            decT_s = wp.tile([128, 2], BF16, name="decT_s", tag="decT_s")
            encb_s = wp.tile([128, 2], F32, name="encb_s", tag="encb_s")
            fobsb_s = wp.tile([128, 2], F32, name="fobsb_s", tag="fobsb_s")
            brz1_s = wp.tile([128, 4], F32, name="brz1_s", tag="brz1_s")
            bhn1_s = wp.tile([128, 2], F32, name="bhn1_s", tag="bhn1_s")
            binb_s = wp.tile([128, 2], F32, name="binb_s", tag="binb_s")
            brz2_s = wp.tile([128, 4], F32, name="brz2_s", tag="brz2_s")
            bhn2_s = wp.tile([128, 2], F32, name="bhn2_s", tag="bhn2_s")
            decb_s = wp.tile([1, 1], F32, name="decb_s", tag="decb_s")

            xT_ap = xT.ap()
            out_ap = out.ap()

            # Prefetch the prologue input tiles ahead of the big weight
            # DMAs (same SP queue) so the first enc matmul starts early.
            nc.sync.dma_start(encT_s[:], encT.ap())
            nc.sync.dma_start(encb_s[:], encb.ap())
            xt_pre = {}
            for t in range(4):
                xt = io.tile([128, R], BF16, name="xt", tag="xt")
                # ACT is also a HWDGE engine: its queue runs in parallel
                # with the SP weight DMAs, halving the ramp.
                nc.scalar.dma_start(xt[:], xT_ap[:, t * R : t * R + R])
                xt_pre[t] = xt
            for tl, dr in [
                (fobsT_s, fobsT), (wf1T_s, wf1T), (brz1_s, brz1),
                (bhn1_s, bhn1), (binb_s, binb), (fobsb_s, fobsb),
                (whhT_s, whhT), (wihT_s, wihT), (brz2_s, brz2),
                (bhn2_s, bhn2), (decT_s, decT), (decb_s, decb),
            ]:
                nc.sync.dma_start(tl[:], dr.ap())

            st = {}

            # ---- A phases: enc -> e; gh1 = Wf1@e; ph = fobs@e ----

            def emitA1(t):
                # enc: e = relu(enc_W @ x + enc_b); one half ACT, one DVE
                if t in xt_pre:
                    xt = xt_pre.pop(t)
                else:
                    r0 = t * R
                    xt = io.tile([128, R], BF16, name="xt", tag="xt")
                    nc.sync.dma_start(xt[:], xT_ap[:, r0 : r0 + R])
                pe = px.tile([128, 1024], F32, name="pe", tag="px")
                for m in (0, 1):
                    mm(pe[:, 512 * m : 512 * m + 512],
                       encT_s[:, 128 * m : 128 * m + 128], xt[:], True, True)
                e = ap.tile([128, 1024], BF16, name="e", tag="e", bufs=5)
                nc.scalar.activation(e[:, S0], pe[:, S0], AF.Relu,
                                     bias=encb_s[:, 0:1])
                nc.vector.tensor_scalar(e[:, S1], pe[:, S1],
                                        encb_s[:, 1:2], 0.0, OP.add, OP.max)
                st[t] = {"e": e}

            def emitA2(t):
                # fobs ph (PSUM, consumed by d1) + gh1 = Wf1 @ e
                e = st[t]["e"]
                # pn allocated before ph: the px FIFO then couples next
                # tile's pe slot to tmp1 (early DVE) instead of d1 (late).
                pn = px.tile([128, 1024], F32, name="pn", tag="px")
                ph = px.tile([128, 1024], F32, name="ph", tag="px")
                prz = [pp.tile([128, 512], F32, name=f"prz{g}", tag="ps")
                       for g in range(4)]
                # k-outer: all k=0 matmuls (reading the ACT-produced e half)
                # run before any k=1 (DVE-produced half), so a late relu on
                # the DVE queue never stalls the PE.
                for k in (0, 1):
                    ek = e[:, 512 * k : 512 * k + 512]
                    for m in (0, 1):
                        mm(ph[:, 512 * m : 512 * m + 512],
                           fobsT_s[:, 256 * k + 128 * m : 256 * k + 128 * m + 128],
                           ek, k == 0, k == 1)
                    for m in (0, 1):
                        mm(pn[:, 512 * m : 512 * m + 512],
                           wf1T_s[:, 768 * k + 512 + 128 * m : 768 * k + 640 + 128 * m],
                           ek, k == 0, k == 1)
                    for g in range(4):
                        mm(prz[g][:],
                           wf1T_s[:, 768 * k + 128 * g : 768 * k + 128 * g + 128],
                           ek, k == 0, k == 1)
                st[t]["ph"] = ph
                st[t]["pn"] = pn
                st[t]["prz"] = prz

            def emitA3a(t):
                # GRU1 sigmoids (ACT)
                prz = st[t].pop("prz")
                rz1 = ap.tile([128, 2048], BF16, name="rz1", tag="rz1", bufs=5)
                for g in range(4):
                    nc.scalar.activation(rz1[:, 512 * g : 512 * g + 512],
                                         prz[g][:], AF.Sigmoid,
                                         bias=brz1_s[:, g : g + 1])
                st[t]["rz1"] = rz1

            def emitA3bHead(t):
                # DVE/ACT: tmp1, n1, d1 (fobs PSUM, bias fused). The tail
                # (m1/h1/S) is emitted one step later, when its inputs are
                # already resident, so it never clogs the DVE queue.
                pn = st[t].pop("pn")
                ph = st[t].pop("ph")
                rz1 = st[t]["rz1"]
                tmp1 = ap.tile([128, 1024], BF16, name="tmp1", tag="tmp1")
                for m, sl in ((0, S0), (1, S1)):
                    nc.vector.scalar_tensor_tensor(
                        tmp1[:, sl], pn[:, sl], bhn1_s[:, m : m + 1],
                        rz1[:, sl], OP.add, OP.mult)
                n1 = ap.tile([128, 1024], BF16, name="n1", tag="n1")
                for m, sl in ((0, S0), (1, S1)):
                    nc.scalar.activation(n1[:, sl], tmp1[:, sl], AF.Tanh,
                                         bias=binb_s[:, m : m + 1])
                d1 = ap.tile([128, 1024], BF16, name="d1", tag="d1")
                for m, sl in ((0, S0), (1, S1)):
                    nc.vector.scalar_tensor_tensor(
                        d1[:, sl], ph[:, sl], fobsb_s[:, m : m + 1],
                        n1[:, sl], OP.add, OP.subtract)
                st[t]["n1"] = n1
                st[t]["d1"] = d1

            def emitA3bTail(t):
                # m1, h1 (DVE); agent pair-add (GpSimd); short reduce (DVE)
                rz1 = st[t].pop("rz1")
                n1 = st[t].pop("n1")
                d1 = st[t].pop("d1")
                m1 = ap.tile([128, 1024], BF16, name="m1", tag="m1")
                nc.vector.tensor_mul(m1[:], rz1[:, 1024:2048], d1[:])
                h1 = ap.tile([128, 1024], BF16, name="h1", tag="h1", bufs=7)
                nc.vector.tensor_add(h1[:], n1[:], m1[:])
                h1v = h1[:].rearrange("p (s two a) -> p two s a", two=2, a=16)
                s2t = ap.tile([128, 512], BF16, name="s2t", tag="s2t")
                nc.gpsimd.tensor_tensor(
                    s2t[:].rearrange("p (s a) -> p s a", a=16),
                    h1v[:, 0:1, :, :].rearrange("p one s a -> p (one s) a"),
                    h1v[:, 1:2, :, :].rearrange("p one s a -> p (one s) a"),
                    OP.add)
                S = ap.tile([128, 32], F32, name="S", tag="S")
                nc.vector.tensor_reduce(
                    S[:], s2t[:].rearrange("p (s a) -> p s a", a=16),
                    mybir.AxisListType.X, OP.add)
                st[t]["h1"] = h1
                st[t]["S"] = S

            def emitA3c(t):
                # comm: c' = (sum_group h1) - h1  (1/A folded into W_ih)
                h1, S = st[t]["h1"], st[t].pop("S")
                cp = ap.tile([128, 1024], BF16, name="cp", tag="cp", bufs=7)
                Sb = S[:].unsqueeze(-1).broadcast_to([128, 32, 32])
                nc.gpsimd.tensor_tensor(
                    cp[:].rearrange("p (s a) -> p s a", a=32), Sb,
                    h1[:].rearrange("p (s a) -> p s a", a=32), OP.subtract)
                st[t]["cp"] = cp

            # ---- B phases: GRU2 ----

            def emitBrz(t):
                h1, cp = st[t]["h1"], st[t]["cp"]
                prz2 = [pp.tile([128, 512], F32, name=f"prz2{g}", tag="ps")
                        for g in range(4)]
                for g in range(4):
                    w0 = 128 * g
                    mm(prz2[g][:], wihT_s[:, w0 : w0 + 128], cp[:, S0], True, False)
                    mm(prz2[g][:], wihT_s[:, 768 + w0 : 768 + w0 + 128], cp[:, S1], False, False)
                    mm(prz2[g][:], whhT_s[:, w0 : w0 + 128], h1[:, S0], False, False)
                    mm(prz2[g][:], whhT_s[:, 768 + w0 : 768 + w0 + 128], h1[:, S1], False, True)
                rz2 = ap.tile([128, 2048], BF16, name="rz2", tag="rz2")
                for g in range(4):
                    nc.scalar.activation(rz2[:, 512 * g : 512 * g + 512],
                                         prz2[g][:], AF.Sigmoid,
                                         bias=brz2_s[:, g : g + 1])
                st[t]["rz2"] = rz2

            def emitBnMM(t):
                h1, cp = st[t]["h1"], st[t]["cp"]
                phn = [pp.tile([128, 512], F32, name=f"phn{m}", tag="ps")
                       for m in (0, 1)]
                for m in (0, 1):
                    for k in (0, 1):
                        mm(phn[m][:],
                           whhT_s[:, 768 * k + 512 + 128 * m : 768 * k + 640 + 128 * m],
                           h1[:, 512 * k : 512 * k + 512], k == 0, k == 1)
                pin = [pp.tile([128, 512], F32, name=f"pin{m}", tag="ps")
                       for m in (0, 1)]
                for m in (0, 1):
                    for k in (0, 1):
                        mm(pin[m][:],
                           wihT_s[:, 768 * k + 512 + 128 * m : 768 * k + 640 + 128 * m],
                           cp[:, 512 * k : 512 * k + 512], k == 0, k == 1)
                st[t]["phn"] = phn
                st[t]["pin"] = pin

            def emitBelemA(t):
                # DVE: tmp2 = (phn + b_hn)*r2 ; s2 = (pin + b_in) + tmp2
                phn = st[t].pop("phn")
                pin = st[t].pop("pin")
                rz2 = st[t]["rz2"]
                tmp2 = ap.tile([128, 1024], BF16, name="tmp2", tag="tmp2")
                for m, sl in ((0, S0), (1, S1)):
                    nc.vector.scalar_tensor_tensor(
                        tmp2[:, sl], phn[m][:], bhn2_s[:, m : m + 1],
                        rz2[:, sl], OP.add, OP.mult)
                s2 = ap.tile([128, 1024], BF16, name="s2", tag="s2")
                for m, sl in ((0, S0), (1, S1)):
                    nc.vector.scalar_tensor_tensor(
                        s2[:, sl], pin[m][:], binb_s[:, m : m + 1],
                        tmp2[:, sl], OP.add, OP.add)
                st[t]["s2"] = s2

            def emitBelemB(t, on_dve=False):
                # n2 (ACT, single 1024-wide); d2/m2 on GpSimd; the final
                # h2 = n2 + m2 rides a SWDGE accumulate-DMA (n2 is written
                # straight into the h2 tile). Drain tiles go full-DVE.
                h1 = st[t]["h1"]
                s2 = st[t].pop("s2")
                rz2 = st[t].pop("rz2")
                if on_dve:
                    n2 = ap.tile([128, 1024], BF16, name="n2", tag="n2")
                    nc.scalar.activation(n2[:], s2[:], AF.Tanh)
                    d2 = ap.tile([128, 1024], BF16, name="d2", tag="d2")
                    nc.vector.tensor_tensor(d2[:], h1[:], n2[:], OP.subtract)
                    m2 = ap.tile([128, 1024], BF16, name="m2", tag="m2")
                    nc.vector.tensor_mul(m2[:], rz2[:, 1024:2048], d2[:])
                    h2 = ap.tile([128, 1024], BF16, name="h2", tag="h2", bufs=5)
                    nc.vector.tensor_add(h2[:], n2[:], m2[:])
                else:
                    h2 = ap.tile([128, 1024], BF16, name="h2", tag="h2", bufs=5)
                    nc.scalar.activation(h2[:], s2[:], AF.Tanh)  # h2 holds n2
                    d2 = ap.tile([128, 1024], BF16, name="d2", tag="d2")
                    nc.gpsimd.tensor_tensor(d2[:], h1[:], h2[:], OP.subtract)
                    m2 = ap.tile([128, 1024], BF16, name="m2", tag="m2")
                    nc.gpsimd.tensor_mul(m2[:], rz2[:, 1024:2048], d2[:])
                    nc.gpsimd.dma_start(h2[:], m2[:], accum_op=OP.add)
                st[t]["h2"] = h2

            def emitC(t):
                h2 = st.pop(t)["h2"]
                r0 = t * R
                pd = pp.tile([1, 512], F32, name="pd", tag="ps")
                mm(pd[:], decT_s[:, 0:1], h2[:, S0], True, False)
                mm(pd[:], decT_s[:, 1:2], h2[:, S1], False, True)
                ot = io.tile([1, 512], F32, name="ot", tag="ot")
                if t % 2 == 0:
                    nc.scalar.activation(ot[:], pd[:], AF.Identity,
                                         bias=decb_s[0:1, 0:1])
                else:
                    nc.vector.tensor_scalar_add(ot[:], pd[:], decb_s[0:1, 0:1])
                nc.sync.dma_start(out_ap[0:1, r0 : r0 + R], ot[:])

            # ---- schedule ----
            # A phases lead B by 3 tiles so the DVE/GpSimd chains (which
            # run up to a tile behind their emission) never gate a PSUM
            # slot reuse or a B-phase matmul; the decoder trails by 3.
            for f in (emitA1, emitA2, emitA3a, emitA3bHead,
                      emitA3bTail, emitA3c):
                f(0)
                f(1)
                f(2)
            for t in range(nt):
                if t + 3 < nt:
                    emitA1(t + 3)
                emitBrz(t)
                emitBnMM(t)
                emitBelemA(t)
                if 3 <= t + 2 < nt:
                    emitA3bTail(t + 2)
                    emitA3c(t + 2)
                if t >= 3:
                    emitC(t - 3)
                if t + 3 < nt:
                    emitA2(t + 3)
                    emitA3a(t + 3)
                    emitA3bHead(t + 3)
                emitBelemB(t, on_dve=(t >= nt - 2))
            for t in range(nt - 3, nt):
                emitC(t)

    nc.compile()
    return nc


def prep_shared(enc_W, enc_b, fobs_W, fobs_b, W_ih, b_ih, W_hh, b_hh, dec_W, dec_b):
    f = np.float32
    whh = W_hh.astype(f)
    wf1 = whh @ fobs_W.astype(f)                 # [768, 256] folded GRU1 weight
    bf1 = whh @ fobs_b.astype(f)                 # [768] folded fobs bias
    whhT = whh.T                                 # [256, 768]
    wf1T = wf1.T                                 # [256, 768]
    wihT = (W_ih / A).T.astype(f)                # [256, 768], 1/A folded in
    bsum = (b_ih + b_hh).astype(f)
    bf = NP_BF16

    def halves(mT):                              # [256, X] -> [128, 2X]
        return np.ascontiguousarray(
            np.concatenate([mT[0:128], mT[128:256]], axis=1)
        ).astype(bf)

    return {
        "encT": np.ascontiguousarray(enc_W.T).astype(bf),                    # [128,256]
        "fobsT": halves(fobs_W.T.astype(f)),                                 # [128,512]
        "wf1T": halves(wf1T),                                                # [128,1536]
        "whhT": halves(whhT),                                                # [128,1536]
        "wihT": halves(wihT),                                                # [128,1536]
        "decT": halves(dec_W.T.astype(f)),                                   # [128,2]
        "encb": np.ascontiguousarray(enc_b.reshape(2, 128).T.astype(f)),
        "fobsb": np.ascontiguousarray(fobs_b.reshape(2, 128).T.astype(f)),
        "brz1": np.ascontiguousarray((bsum[0:512] + bf1[0:512]).reshape(4, 128).T),
        "bhn1": np.ascontiguousarray(
            (b_hh[512:768] + bf1[512:768]).reshape(2, 128).T.astype(f)
        ),
        "binb": np.ascontiguousarray(b_ih[512:768].reshape(2, 128).T.astype(f)),
        "brz2": np.ascontiguousarray(bsum[0:512].reshape(4, 128).T),
        "bhn2": np.ascontiguousarray(b_hh[512:768].reshape(2, 128).T.astype(f)),
        "decb": dec_b.reshape(1, 1).astype(f),
    }


_NC_CACHE = {}


def _get_nc(n_rows):
    if n_rows not in _NC_CACHE:
        _NC_CACHE[n_rows] = build_nc(n_rows)
    return _NC_CACHE[n_rows]


def run(inputs, trace=False):
    """Shard, run on 8 cores, gather. Returns (out [B,A,1] f32, results)."""
    obs = np.asarray(inputs["obs"], dtype=np.float32)
    shared = prep_shared(
        np.asarray(inputs["enc_W"]), np.asarray(inputs["enc_b"]),
        np.asarray(inputs["fobs_W"]), np.asarray(inputs["fobs_b"]),
        np.asarray(inputs["W_ih"]), np.asarray(inputs["b_ih"]),
        np.asarray(inputs["W_hh"]), np.asarray(inputs["b_hh"]),
        np.asarray(inputs["dec_W"]), np.asarray(inputs["dec_b"]),
    )
    in_maps = []
    for c in range(NCORES):
        xT = np.ascontiguousarray(
            obs[c * B_LOC : (c + 1) * B_LOC].reshape(N_LOC, D).T
        ).astype(NP_BF16)
        in_maps.append({"xT": xT, **shared})

    nc = _get_nc(N_LOC)
    res = run_bass_kernel_spmd(nc, in_maps, core_ids=list(range(NCORES)), trace=trace)
    outs = [res.results[c]["out"].reshape(N_LOC) for c in range(NCORES)]
    full = np.concatenate(outs).reshape(B, A, 1).astype(np.float32)
    return full, res


def kernel(**inputs):
    out, _ = run(inputs, trace=False)
    return out
